# revision 33
# baseline (speedup 1.0000x reference)
"""Bahdanau-attention Trainium2 kernel (nn_Attention_3616362463521).

Math (per batch b):
    pre[k]      = sum_h hidden[b,h] * wh[k,h] + bias[k]          (wh = attn_w[:, :H])
    energy[k,s] = tanh(sum_h enc[b,s,h] * we[k,h] + pre[k])      (we = attn_w[:, H:])
    scores[s]   = sum_k v[k] * energy[k,s]
    w[s]        = softmax(scores)        (no max-shift: |scores| <= ||v||_1 ~ 16)
    ctx[h]      = sum_s w[s] * enc[b,s,h]

Sharding: data-parallel over batch, 4 batches per core on 8 cores; weights
replicated. Matmul inputs are bf16 (fp32 PSUM accumulation), softmax in fp32.

B=32, S=2048, H=1024 hardcoded.
"""

import numpy as np
import ml_dtypes

B, S, H = 32, 2048, 1024
NCORES = 8
BL = B // NCORES          # batches per core
P = 128
HC = H // P               # 8 h-chunks
KC = H // P               # 8 k-chunks
ST = S // P               # 16 s-tiles
SC = S // 512             # 4 s-chunks of 512


def build_bass():
    import bass_rust
    import concourse.mybir as mybir
    import concourse.tile as tile
    from concourse import bacc
    from concourse.masks import make_identity

    f32 = mybir.dt.float32
    bf16 = mybir.dt.bfloat16
    AF = mybir.ActivationFunctionType

    nc = bacc.Bacc("TRN2", target_bir_lowering=False)

    encn_d = nc.dram_tensor("encn", [BL, SC, P, 4, H], bf16, kind="ExternalInput")
    enct_d = nc.dram_tensor("enct", [BL, SC, P, HC, 512], bf16, kind="ExternalInput")
    wet_d = nc.dram_tensor("wet", [P, HC, H], bf16, kind="ExternalInput")
    wht_d = nc.dram_tensor("wht", [P, HC, H], bf16, kind="ExternalInput")
    hid_d = nc.dram_tensor("hid", [P, HC, BL], bf16, kind="ExternalInput")
    biasc_d = nc.dram_tensor("biasc", [P, KC], f32, kind="ExternalInput")
    vcol_d = nc.dram_tensor("vcol", [P, KC], f32, kind="ExternalInput")
    ctx_d = nc.dram_tensor("ctx", [BL, H], f32, kind="ExternalOutput")
    wout_d = nc.dram_tensor("wout", [BL, S], f32, kind="ExternalOutput")

    with tile.TileContext(nc) as tc:
        with (
            tc.tile_pool(name="singles", bufs=1) as singles,
            tc.tile_pool(name="nat", bufs=4) as nat_pool,
            tc.tile_pool(name="enct", bufs=3) as enct_pool,
            tc.tile_pool(name="tanh", bufs=2) as tanh_pool,
            tc.tile_pool(name="sm", bufs=2) as sm_pool,
            tc.tile_pool(name="outs", bufs=2) as out_pool,
            tc.tile_pool(name="pe", bufs=3, space="PSUM") as pe_pool,
            tc.tile_pool(name="pctx", bufs=2, space="PSUM") as pctx,
            tc.tile_pool(name="psmall", bufs=3, space="PSUM") as psmall,
        ):
            # ---- first chunk loads go out first so energy can start early
            eT0 = enct_pool.tile([P, HC, 512], bf16, tag="encT")
            ld_eT0 = nc.sync.dma_start(out=eT0, in_=enct_d.ap()[0][0])
            sub0 = nat_pool.tile([P, 4, H], bf16, tag="enc_nat")
            ld_sub0 = nc.gpsimd.dma_start(out=sub0, in_=encn_d.ap()[0][0])
            bass_rust.add_dep_helper(
                ld_sub0.ins, ld_eT0.ins, sync=True,
                reason="stagger: sub0 after critical eT0",
            )

            # ---- constants ----
            wet_sb = singles.tile([P, HC, H], bf16)
            wht_sb = singles.tile([P, HC, H], bf16)
            hid_sb = singles.tile([P, HC, BL], bf16)
            biasc_sb = singles.tile([P, KC], f32)
            vcol_sb = singles.tile([P, KC], f32)
            nc.sync.dma_start(out=hid_sb, in_=hid_d.ap())
            nc.sync.dma_start(out=biasc_sb, in_=biasc_d.ap())
            nc.sync.dma_start(out=vcol_sb, in_=vcol_d.ap())
            nc.sync.dma_start(out=wet_sb, in_=wet_d.ap())
            ld_wht = nc.sync.dma_start(out=wht_sb, in_=wht_d.ap())
            bass_rust.add_dep_helper(
                ld_wht.ins, ld_eT0.ins, sync=True,
                reason="stagger: wht after critical eT0",
            )

            ident_f32 = singles.tile([P, P], f32)
            make_identity(nc, ident_f32)
            ones_col = singles.tile([P, 1], f32)
            ones_col_bf = singles.tile([P, 1], bf16)
            ones_row = singles.tile([1, P], f32)
            nc.vector.memset(ones_col, 1.0)
            nc.vector.memset(ones_col_bf, 1.0)
            nc.vector.memset(ones_row, 1.0)

            # ---- pre projection: pre[k, b] = sum_h wh[k,h] hid[b,h] + bias[k] ----
            pre_sb = singles.tile([P, KC, BL], f32)
            for kc in range(KC):
                ps_pre = psmall.tile([P, BL], f32, tag="small")
                for hc in range(HC):
                    nc.tensor.matmul(
                        ps_pre,
                        lhsT=wht_sb[:, hc, kc * P:(kc + 1) * P],
                        rhs=hid_sb[:, hc, :],
                        start=(hc == 0),
                        stop=(hc == HC - 1),
                    )
                nc.scalar.activation(
                    out=pre_sb[:, kc, :], in_=ps_pre, func=AF.Identity,
                    bias=biasc_sb[:, kc:kc + 1], scale=1.0,
                )

            prev_crit = ld_eT0
            for b in range(BL):
                exp_bf = sm_pool.tile([P, ST], bf16, tag="exp_bf")
                zparts = []
                # Unnormalized context accumulators (whole batch).
                ps_c0 = pctx.tile([1, 512], f32, tag="ctx")
                ps_c1 = pctx.tile([1, 512], f32, tag="ctx")

                for sc in range(SC):
                    # ---- load enc s-chunk, natural + transposed layouts ----
                    if b == 0 and sc == 0:
                        sub, eT = sub0, eT0
                    else:
                        sub = nat_pool.tile([P, 4, H], bf16, tag="enc_nat")
                        ld_s = nc.gpsimd.dma_start(out=sub, in_=encn_d.ap()[b][sc])
                        eT = enct_pool.tile([P, HC, 512], bf16, tag="encT")
                        ld_t = nc.sync.dma_start(out=eT, in_=enct_d.ap()[b][sc])
                        if b == 0:
                            for ld in (ld_s, ld_t):
                                bass_rust.add_dep_helper(
                                    ld.ins, prev_crit.ins, sync=True,
                                    reason="stagger: chain batch-0 prefetches",
                                )
                            prev_crit = ld_t

                    # ---- energy + tanh ----
                    tanh_sb = tanh_pool.tile([P, KC, 512], bf16, tag="tanh")
                    for kc in range(KC):
                        ps_e = pe_pool.tile([P, 512], f32, tag="ps_e")
                        for hc in range(HC):
                            nc.tensor.matmul(
                                ps_e,
                                lhsT=wet_sb[:, hc, kc * P:(kc + 1) * P],
                                rhs=eT[:, hc, :],
                                start=(hc == 0),
                                stop=(hc == HC - 1),
                            )
                        nc.scalar.activation(
                            out=tanh_sb[:, kc, :], in_=ps_e, func=AF.Tanh,
                            bias=pre_sb[:, kc, b:b + 1], scale=1.0,
                        )

                    # ---- scores for this chunk ----
                    # v-weighted sum over k: multiplies + adds on VectorE,
                    # 128-partition reduction as one matmul with a ones column.
                    acc = None
                    for kc in range(KC):
                        t = sm_pool.tile([P, 512], bf16, tag=f"vt{kc % 2}")
                        nc.vector.tensor_scalar_mul(
                            t, tanh_sb[:, kc, :], vcol_sb[:, kc:kc + 1]
                        )
                        if acc is None:
                            acc = t
                        else:
                            a = sm_pool.tile([P, 512], bf16, tag=f"va{kc % 2}")
                            nc.vector.tensor_add(a, acc, t)
                            acc = a
                    # ---- k-partition reduce straight to column form ----
                    # scores_col[s', i] = sum_k acc[k, i*128+s']
                    ps_sc4 = psmall.tile([P, 4], f32, tag="small")
                    for i in range(4):
                        nc.tensor.matmul(
                            ps_sc4[:, i:i + 1],
                            lhsT=acc[:, i * P:(i + 1) * P],
                            rhs=ones_col_bf,
                            start=True,
                            stop=True,
                        )
                    zp = sm_pool.tile([P, 1], f32, tag=f"zpart{sc}")
                    nc.scalar.activation(
                        out=exp_bf[:, sc * 4:(sc + 1) * 4], in_=ps_sc4,
                        func=AF.Exp, accum_out=zp,
                    )
                    zparts.append(zp)

                    # ---- unnormalized context accumulation ----
                    for i in range(4):
                        for hh, ps_c in enumerate((ps_c0, ps_c1)):
                            nc.tensor.matmul(
                                ps_c,
                                lhsT=exp_bf[:, sc * 4 + i:sc * 4 + i + 1],
                                rhs=sub[:, i, hh * 512:(hh + 1) * 512],
                                start=(sc == 0 and i == 0),
                                stop=(sc == SC - 1 and i == 3),
                            )

                # ---- Z and outputs ----
                zcol = sm_pool.tile([P, 1], f32, tag="zcol")
                nc.vector.tensor_add(zcol, zparts[0], zparts[1])
                nc.vector.tensor_add(zcol, zcol, zparts[2])
                nc.vector.tensor_add(zcol, zcol, zparts[3])
                ps_z = psmall.tile([1, 1], f32, tag="small")
                nc.tensor.matmul(ps_z, lhsT=zcol, rhs=ones_col, start=True, stop=True)
                rz = sm_pool.tile([1, 1], f32, tag="rz")
                nc.vector.reciprocal(rz, ps_z)

                ctx_sb = out_pool.tile([1, H], f32, tag="ctx_sb")
                nc.vector.tensor_scalar_mul(ctx_sb[:, :512], ps_c0, rz)
                nc.vector.tensor_scalar_mul(ctx_sb[:, 512:], ps_c1, rz)
                nc.scalar.dma_start(out=ctx_d.ap()[b:b + 1, :], in_=ctx_sb)

                # attention weights out: w = exp * (1/Z), row layout via transpose
                ps_rz = psmall.tile([P, 1], f32, tag="small")
                nc.tensor.matmul(ps_rz, lhsT=ones_row, rhs=rz, start=True, stop=True)
                rzb = sm_pool.tile([P, 1], f32, tag="rzb")
                nc.vector.tensor_copy(out=rzb, in_=ps_rz)
                wcol_f32 = sm_pool.tile([P, ST], f32, tag="wcol_f32")
                nc.vector.tensor_scalar_mul(wcol_f32, exp_bf, rzb)
                ps_w = psmall.tile([ST, P], f32, tag="small")
                nc.tensor.transpose(ps_w, wcol_f32, ident_f32)
                wrow_sb = out_pool.tile([ST, P], f32, tag="wrow_sb")
                nc.vector.tensor_copy(out=wrow_sb, in_=ps_w)
                nc.scalar.dma_start(
                    out=wout_d.ap()[b].rearrange("(st p) -> st p", p=P), in_=wrow_sb
                )

    nc.finalize()
    return nc


def _prep_inputs(hidden, encoder_outputs, attn_w, attn_b, v_w):
    """Host-side weight layout prep + per-core input maps."""
    bf16 = ml_dtypes.bfloat16
    hidden = np.asarray(hidden, dtype=np.float32)
    encoder_outputs = np.ascontiguousarray(np.asarray(encoder_outputs, dtype=np.float32))
    attn_w = np.asarray(attn_w, dtype=np.float32)
    attn_b = np.asarray(attn_b, dtype=np.float32)
    v_w = np.asarray(v_w, dtype=np.float32)

    wh = attn_w[:, :H]           # [k, h]
    we = attn_w[:, H:]           # [k, h]
    # wet[p, hc, k] = we[k, hc*128 + p]
    wet = np.ascontiguousarray(
        we.T.reshape(HC, P, H).transpose(1, 0, 2).astype(bf16)
    )
    wht = np.ascontiguousarray(
        wh.T.reshape(HC, P, H).transpose(1, 0, 2).astype(bf16)
    )
    biasc = np.ascontiguousarray(attn_b.reshape(KC, P).T.astype(np.float32))
    vcol = np.ascontiguousarray(v_w[0].reshape(KC, P).T.astype(np.float32))

    in_maps = []
    for c in range(NCORES):
        b0 = c * BL
        # hid[p, hc, b] = hidden[0, b0+b, hc*128+p]
        hid = np.ascontiguousarray(
            hidden[0, b0:b0 + BL].T.reshape(HC, P, BL).transpose(1, 0, 2).astype(bf16)
        )
        enc_c = encoder_outputs[b0:b0 + BL]
        # encn[b, sc, p, i, h] = enc[b, sc*512 + i*128 + p, h]
        encn = np.ascontiguousarray(
            enc_c.reshape(BL, SC, 4, P, H).transpose(0, 1, 3, 2, 4).astype(bf16)
        )
        # enct[b, sc, p, hc, s'] = enc[b, sc*512 + s', hc*128 + p]
        enct = np.ascontiguousarray(
            enc_c.transpose(0, 2, 1).reshape(BL, HC, P, SC, 512)
            .transpose(0, 3, 2, 1, 4).astype(bf16)
        )
        in_maps.append({
            "encn": encn,
            "enct": enct,
            "wet": wet,
            "wht": wht,
            "hid": hid,
            "biasc": biasc,
            "vcol": vcol,
        })
    return in_maps


_NC_CACHE = {}


def _ensure_ntff_hook():
    """The trimmed container lacks antenv.axon_hooks, so the boot-time NTFF
    profile hook registration silently degraded.  Recreate the registry module
    and register the ctypes hook against the axon PJRT .so."""
    import sys
    import types

    try:
        import antenv.axon_hooks  # noqa: F401
        return
    except ImportError:
        pass
    import antenv

    mod = types.ModuleType("antenv.axon_hooks")
    _reg = {"hook": None}
    mod.set_axon_ntff_profile_hook = lambda h: _reg.__setitem__("hook", h)
    mod.get_axon_ntff_profile_hook = lambda: _reg["hook"]
    sys.modules["antenv.axon_hooks"] = mod
    antenv.axon_hooks = mod
    try:
        from trn_agent_boot.trn_boot import _ntff_profile_via_ctypes

        hook = _ntff_profile_via_ctypes("/opt/axon/libaxon_pjrt.so")
        if hook is not None:
            mod.set_axon_ntff_profile_hook(hook)
    except Exception as e:  # pragma: no cover
        print("ntff hook setup failed:", e)


def run(hidden, encoder_outputs, attn_w, attn_b, v_w, trace=False, tmpdir=None,
        trace_cores=None):
    from concourse import bass_utils

    if trace:
        _ensure_ntff_hook()
        # No bucket access in this container; keep artifacts local.
        bass_utils.upload_artifacts = lambda d: f"file://{d}"

    if "nc" not in _NC_CACHE:
        _NC_CACHE["nc"] = build_bass()
    nc = _NC_CACHE["nc"]
    in_maps = _prep_inputs(hidden, encoder_outputs, attn_w, attn_b, v_w)
    res = bass_utils.run_bass_kernel_spmd(
        nc, in_maps, core_ids=list(range(NCORES)), trace=trace, tmpdir=tmpdir,
        trace_cores=trace_cores,
    )
    ctx = np.concatenate([r["ctx"] for r in res.results], axis=0)     # [B, H]
    wout = np.concatenate([r["wout"] for r in res.results], axis=0)   # [B, S]
    context = ctx.reshape(B, 1, H).astype(np.float32)
    attn_weights = wout.astype(np.float32)
    return (context, attn_weights), res


def kernel(hidden, encoder_outputs, attn_w, attn_b, v_w):
    (context, attn_weights), _ = run(hidden, encoder_outputs, attn_w, attn_b, v_w)
    return context, attn_weights


# revision 34
# speedup vs baseline: 1.0314x; 1.0314x over previous
"""Bahdanau-attention Trainium2 kernel (nn_Attention_3616362463521).

Math (per batch b):
    pre[k]      = sum_h hidden[b,h] * wh[k,h] + bias[k]          (wh = attn_w[:, :H])
    energy[k,s] = tanh(sum_h enc[b,s,h] * we[k,h] + pre[k])      (we = attn_w[:, H:])
    scores[s]   = sum_k v[k] * energy[k,s]
    w[s]        = softmax(scores)        (no max-shift: |scores| <= ||v||_1 ~ 16)
    ctx[h]      = sum_s w[s] * enc[b,s,h]

Sharding: data-parallel over batch, 4 batches per core on 8 cores; weights
replicated. Matmul inputs are bf16 (fp32 PSUM accumulation), softmax in fp32.

B=32, S=2048, H=1024 hardcoded.
"""

import numpy as np
import ml_dtypes

B, S, H = 32, 2048, 1024
NCORES = 8
BL = B // NCORES          # batches per core
P = 128
HC = H // P               # 8 h-chunks
KC = H // P               # 8 k-chunks
ST = S // P               # 16 s-tiles
SC = S // 512             # 4 s-chunks of 512


def build_bass():
    import bass_rust
    import concourse.mybir as mybir
    import concourse.tile as tile
    from concourse import bacc
    from concourse.masks import make_identity

    f32 = mybir.dt.float32
    bf16 = mybir.dt.bfloat16
    AF = mybir.ActivationFunctionType

    nc = bacc.Bacc("TRN2", target_bir_lowering=False)

    encn_d = nc.dram_tensor("encn", [BL, SC, P, 4, H], bf16, kind="ExternalInput")
    enct_d = nc.dram_tensor("enct", [BL, SC, P, HC, 512], bf16, kind="ExternalInput")
    wet_d = nc.dram_tensor("wet", [P, HC, H], bf16, kind="ExternalInput")
    wht_d = nc.dram_tensor("wht", [P, HC, H], bf16, kind="ExternalInput")
    hid_d = nc.dram_tensor("hid", [P, HC, BL], bf16, kind="ExternalInput")
    biasc_d = nc.dram_tensor("biasc", [P, KC], f32, kind="ExternalInput")
    vcol_d = nc.dram_tensor("vcol", [P, KC], f32, kind="ExternalInput")
    ctx_d = nc.dram_tensor("ctx", [BL, H], f32, kind="ExternalOutput")
    wout_d = nc.dram_tensor("wout", [BL, S], f32, kind="ExternalOutput")

    with tile.TileContext(nc) as tc:
        with (
            tc.tile_pool(name="singles", bufs=1) as singles,
            tc.tile_pool(name="nat", bufs=4) as nat_pool,
            tc.tile_pool(name="enct", bufs=3) as enct_pool,
            tc.tile_pool(name="tanh", bufs=2) as tanh_pool,
            tc.tile_pool(name="sm", bufs=2) as sm_pool,
            tc.tile_pool(name="outs", bufs=2) as out_pool,
            tc.tile_pool(name="pe", bufs=3, space="PSUM") as pe_pool,
            tc.tile_pool(name="pctx", bufs=2, space="PSUM") as pctx,
            tc.tile_pool(name="psmall", bufs=3, space="PSUM") as psmall,
        ):
            # ---- startup ordering: wht/hid first so the pre projection
            # fills the PE-idle window while eT0/wet stream in; then eT0,
            # then wet; everything else staggered behind eT0.
            wet_sb = singles.tile([P, HC, H], bf16)
            wht_sb = singles.tile([P, HC, H], bf16)
            hid_sb = singles.tile([P, HC, BL], bf16)
            biasc_sb = singles.tile([P, KC], f32)
            vcol_sb = singles.tile([P, KC], f32)
            nc.sync.dma_start(out=hid_sb, in_=hid_d.ap())
            nc.sync.dma_start(out=biasc_sb, in_=biasc_d.ap())
            nc.sync.dma_start(out=vcol_sb, in_=vcol_d.ap())
            nc.sync.dma_start(out=wht_sb, in_=wht_d.ap())
            eT0 = enct_pool.tile([P, HC, 512], bf16, tag="encT")
            ld_eT0 = nc.sync.dma_start(out=eT0, in_=enct_d.ap()[0][0])
            nc.sync.dma_start(out=wet_sb, in_=wet_d.ap())
            sub0 = nat_pool.tile([P, 4, H], bf16, tag="enc_nat")
            ld_sub0 = nc.gpsimd.dma_start(out=sub0, in_=encn_d.ap()[0][0])
            bass_rust.add_dep_helper(
                ld_sub0.ins, ld_eT0.ins, sync=True,
                reason="stagger: sub0 after critical eT0",
            )

            ident_f32 = singles.tile([P, P], f32)
            make_identity(nc, ident_f32)
            ones_col = singles.tile([P, 1], f32)
            ones_col_bf = singles.tile([P, 1], bf16)
            ones_row = singles.tile([1, P], f32)
            nc.vector.memset(ones_col, 1.0)
            nc.vector.memset(ones_col_bf, 1.0)
            nc.vector.memset(ones_row, 1.0)

            # ---- pre projection: pre[k, b] = sum_h wh[k,h] hid[b,h] + bias[k] ----
            pre_sb = singles.tile([P, KC, BL], f32)
            for kc in range(KC):
                ps_pre = psmall.tile([P, BL], f32, tag="small")
                for hc in range(HC):
                    nc.tensor.matmul(
                        ps_pre,
                        lhsT=wht_sb[:, hc, kc * P:(kc + 1) * P],
                        rhs=hid_sb[:, hc, :],
                        start=(hc == 0),
                        stop=(hc == HC - 1),
                    )
                nc.scalar.activation(
                    out=pre_sb[:, kc, :], in_=ps_pre, func=AF.Identity,
                    bias=biasc_sb[:, kc:kc + 1], scale=1.0,
                )

            prev_crit = ld_eT0
            for b in range(BL):
                exp_bf = sm_pool.tile([P, ST], bf16, tag="exp_bf")
                zparts = []
                # Unnormalized context accumulators (whole batch).
                ps_c0 = pctx.tile([1, 512], f32, tag="ctx")
                ps_c1 = pctx.tile([1, 512], f32, tag="ctx")

                for sc in range(SC):
                    # ---- load enc s-chunk, natural + transposed layouts ----
                    if b == 0 and sc == 0:
                        sub, eT = sub0, eT0
                    else:
                        sub = nat_pool.tile([P, 4, H], bf16, tag="enc_nat")
                        ld_s = nc.gpsimd.dma_start(out=sub, in_=encn_d.ap()[b][sc])
                        eT = enct_pool.tile([P, HC, 512], bf16, tag="encT")
                        ld_t = nc.sync.dma_start(out=eT, in_=enct_d.ap()[b][sc])
                        if b == 0:
                            for ld in (ld_s, ld_t):
                                bass_rust.add_dep_helper(
                                    ld.ins, prev_crit.ins, sync=True,
                                    reason="stagger: chain batch-0 prefetches",
                                )
                            prev_crit = ld_t

                    # ---- energy + tanh ----
                    tanh_sb = tanh_pool.tile([P, KC, 512], bf16, tag="tanh")
                    for kc in range(KC):
                        ps_e = pe_pool.tile([P, 512], f32, tag="ps_e")
                        for hc in range(HC):
                            nc.tensor.matmul(
                                ps_e,
                                lhsT=wet_sb[:, hc, kc * P:(kc + 1) * P],
                                rhs=eT[:, hc, :],
                                start=(hc == 0),
                                stop=(hc == HC - 1),
                            )
                        nc.scalar.activation(
                            out=tanh_sb[:, kc, :], in_=ps_e, func=AF.Tanh,
                            bias=pre_sb[:, kc, b:b + 1], scale=1.0,
                        )

                    # ---- scores for this chunk ----
                    # v-weighted sum over k: multiplies + adds on VectorE,
                    # 128-partition reduction as one matmul with a ones column.
                    acc = None
                    for kc in range(KC):
                        t = sm_pool.tile([P, 512], bf16, tag=f"vt{kc % 2}")
                        nc.vector.tensor_scalar_mul(
                            t, tanh_sb[:, kc, :], vcol_sb[:, kc:kc + 1]
                        )
                        if acc is None:
                            acc = t
                        else:
                            a = sm_pool.tile([P, 512], bf16, tag=f"va{kc % 2}")
                            nc.vector.tensor_add(a, acc, t)
                            acc = a
                    # ---- k-partition reduce straight to column form ----
                    # scores_col[s', i] = sum_k acc[k, i*128+s']
                    ps_sc4 = psmall.tile([P, 4], f32, tag="small")
                    for i in range(4):
                        nc.tensor.matmul(
                            ps_sc4[:, i:i + 1],
                            lhsT=acc[:, i * P:(i + 1) * P],
                            rhs=ones_col_bf,
                            start=True,
                            stop=True,
                        )
                    zp = sm_pool.tile([P, 1], f32, tag=f"zpart{sc}")
                    nc.scalar.activation(
                        out=exp_bf[:, sc * 4:(sc + 1) * 4], in_=ps_sc4,
                        func=AF.Exp, accum_out=zp,
                    )
                    zparts.append(zp)

                    # ---- unnormalized context accumulation ----
                    for i in range(4):
                        for hh, ps_c in enumerate((ps_c0, ps_c1)):
                            nc.tensor.matmul(
                                ps_c,
                                lhsT=exp_bf[:, sc * 4 + i:sc * 4 + i + 1],
                                rhs=sub[:, i, hh * 512:(hh + 1) * 512],
                                start=(sc == 0 and i == 0),
                                stop=(sc == SC - 1 and i == 3),
                            )

                # ---- Z and outputs ----
                zcol = sm_pool.tile([P, 1], f32, tag="zcol")
                nc.vector.tensor_add(zcol, zparts[0], zparts[1])
                nc.vector.tensor_add(zcol, zcol, zparts[2])
                nc.vector.tensor_add(zcol, zcol, zparts[3])
                ps_z = psmall.tile([1, 1], f32, tag="small")
                nc.tensor.matmul(ps_z, lhsT=zcol, rhs=ones_col, start=True, stop=True)
                rz = sm_pool.tile([1, 1], f32, tag="rz")
                nc.vector.reciprocal(rz, ps_z)

                ctx_sb = out_pool.tile([1, H], f32, tag="ctx_sb")
                nc.vector.tensor_scalar_mul(ctx_sb[:, :512], ps_c0, rz)
                nc.vector.tensor_scalar_mul(ctx_sb[:, 512:], ps_c1, rz)
                nc.scalar.dma_start(out=ctx_d.ap()[b:b + 1, :], in_=ctx_sb)

                # attention weights out: w = exp * (1/Z), row layout via transpose
                ps_rz = psmall.tile([P, 1], f32, tag="small")
                nc.tensor.matmul(ps_rz, lhsT=ones_row, rhs=rz, start=True, stop=True)
                rzb = sm_pool.tile([P, 1], f32, tag="rzb")
                nc.vector.tensor_copy(out=rzb, in_=ps_rz)
                wcol_f32 = sm_pool.tile([P, ST], f32, tag="wcol_f32")
                nc.vector.tensor_scalar_mul(wcol_f32, exp_bf, rzb)
                ps_w = psmall.tile([ST, P], f32, tag="small")
                nc.tensor.transpose(ps_w, wcol_f32, ident_f32)
                wrow_sb = out_pool.tile([ST, P], f32, tag="wrow_sb")
                nc.vector.tensor_copy(out=wrow_sb, in_=ps_w)
                nc.scalar.dma_start(
                    out=wout_d.ap()[b].rearrange("(st p) -> st p", p=P), in_=wrow_sb
                )

    nc.finalize()
    return nc


def _prep_inputs(hidden, encoder_outputs, attn_w, attn_b, v_w):
    """Host-side weight layout prep + per-core input maps."""
    bf16 = ml_dtypes.bfloat16
    hidden = np.asarray(hidden, dtype=np.float32)
    encoder_outputs = np.ascontiguousarray(np.asarray(encoder_outputs, dtype=np.float32))
    attn_w = np.asarray(attn_w, dtype=np.float32)
    attn_b = np.asarray(attn_b, dtype=np.float32)
    v_w = np.asarray(v_w, dtype=np.float32)

    wh = attn_w[:, :H]           # [k, h]
    we = attn_w[:, H:]           # [k, h]
    # wet[p, hc, k] = we[k, hc*128 + p]
    wet = np.ascontiguousarray(
        we.T.reshape(HC, P, H).transpose(1, 0, 2).astype(bf16)
    )
    wht = np.ascontiguousarray(
        wh.T.reshape(HC, P, H).transpose(1, 0, 2).astype(bf16)
    )
    biasc = np.ascontiguousarray(attn_b.reshape(KC, P).T.astype(np.float32))
    vcol = np.ascontiguousarray(v_w[0].reshape(KC, P).T.astype(np.float32))

    in_maps = []
    for c in range(NCORES):
        b0 = c * BL
        # hid[p, hc, b] = hidden[0, b0+b, hc*128+p]
        hid = np.ascontiguousarray(
            hidden[0, b0:b0 + BL].T.reshape(HC, P, BL).transpose(1, 0, 2).astype(bf16)
        )
        enc_c = encoder_outputs[b0:b0 + BL]
        # encn[b, sc, p, i, h] = enc[b, sc*512 + i*128 + p, h]
        encn = np.ascontiguousarray(
            enc_c.reshape(BL, SC, 4, P, H).transpose(0, 1, 3, 2, 4).astype(bf16)
        )
        # enct[b, sc, p, hc, s'] = enc[b, sc*512 + s', hc*128 + p]
        enct = np.ascontiguousarray(
            enc_c.transpose(0, 2, 1).reshape(BL, HC, P, SC, 512)
            .transpose(0, 3, 2, 1, 4).astype(bf16)
        )
        in_maps.append({
            "encn": encn,
            "enct": enct,
            "wet": wet,
            "wht": wht,
            "hid": hid,
            "biasc": biasc,
            "vcol": vcol,
        })
    return in_maps


_NC_CACHE = {}


def _ensure_ntff_hook():
    """The trimmed container lacks antenv.axon_hooks, so the boot-time NTFF
    profile hook registration silently degraded.  Recreate the registry module
    and register the ctypes hook against the axon PJRT .so."""
    import sys
    import types

    try:
        import antenv.axon_hooks  # noqa: F401
        return
    except ImportError:
        pass
    import antenv

    mod = types.ModuleType("antenv.axon_hooks")
    _reg = {"hook": None}
    mod.set_axon_ntff_profile_hook = lambda h: _reg.__setitem__("hook", h)
    mod.get_axon_ntff_profile_hook = lambda: _reg["hook"]
    sys.modules["antenv.axon_hooks"] = mod
    antenv.axon_hooks = mod
    try:
        from trn_agent_boot.trn_boot import _ntff_profile_via_ctypes

        hook = _ntff_profile_via_ctypes("/opt/axon/libaxon_pjrt.so")
        if hook is not None:
            mod.set_axon_ntff_profile_hook(hook)
    except Exception as e:  # pragma: no cover
        print("ntff hook setup failed:", e)


def run(hidden, encoder_outputs, attn_w, attn_b, v_w, trace=False, tmpdir=None,
        trace_cores=None):
    from concourse import bass_utils

    if trace:
        _ensure_ntff_hook()
        # No bucket access in this container; keep artifacts local.
        bass_utils.upload_artifacts = lambda d: f"file://{d}"

    if "nc" not in _NC_CACHE:
        _NC_CACHE["nc"] = build_bass()
    nc = _NC_CACHE["nc"]
    in_maps = _prep_inputs(hidden, encoder_outputs, attn_w, attn_b, v_w)
    res = bass_utils.run_bass_kernel_spmd(
        nc, in_maps, core_ids=list(range(NCORES)), trace=trace, tmpdir=tmpdir,
        trace_cores=trace_cores,
    )
    ctx = np.concatenate([r["ctx"] for r in res.results], axis=0)     # [B, H]
    wout = np.concatenate([r["wout"] for r in res.results], axis=0)   # [B, S]
    context = ctx.reshape(B, 1, H).astype(np.float32)
    attn_weights = wout.astype(np.float32)
    return (context, attn_weights), res


def kernel(hidden, encoder_outputs, attn_w, attn_b, v_w):
    (context, attn_weights), _ = run(hidden, encoder_outputs, attn_w, attn_b, v_w)
    return context, attn_weights
